# revision 6
# baseline (speedup 1.0000x reference)
"""ExceptionalEGNN (E8 Lie-algebra GNN message passing) on 8 Trainium2 cores.

Strategy (graph/data parallel):
  - Nodes are split into 8 contiguous shards of 2500; each core owns the
    edges whose dst lies in its shard, bucketed further into 512-node
    chunks so the scatter-add is a fixed one-hot matmul structure (SPMD
    uniform across cores; padding edges carry dstloc=-1e9 -> zero one-hot).
  - Full node state h lives replicated in DRAM (h_full), refreshed by an
    AllGather of the 2500-row shards after every layer update.
  - Edge gathers use indirect DMA (128 rows / instruction); gathered
    tiles are PE-transposed to feature-major; all matmuls run in fp32r
    (full PE speed at free dim 512) with fp32 PSUM accumulation.
  - The Lie bracket is computed densely:  br = (hs Gi) * (hd Gj) @ Skc,
    folded into the message MLP:  m = silu(hs W1a + T Wz + b1) W2 + b2,
    with Wz = diag(sc_c) Sk W1b precomputed on host from the tiny tables.
  - Aggregation: agg^T += m^T per node-chunk via one-hot matmuls in PSUM.
  - Final Killing-form scalar, mean-pool (one-hot matmul + AllReduce of
    the 249x128 partial), and output MLP run on-device; core 0's output
    is returned.
"""

import numpy as np

N, IN, D, HID, OUT, NL, NG = 20000, 128, 248, 256, 16, 4, 128
NNZ, NB = 512, 1024
NCORES = 8
S = N // NCORES            # 2500 nodes per core
CH = 512                   # node chunk (one-hot matmul width)
NCH = (S + CH - 1) // CH   # 5 chunks (last has 452 nodes)
NT = (S + 127) // 128      # 20 node tiles per core
GG = 512                   # edges per group
DP = 256                   # padded feature row in DRAM h tables


def _split_multi_waits(nc, mybir, bass_rust):
    """TRN2 allows one sync-wait per instruction; hoist extras onto NoOps."""
    n = 0
    for func in nc.m.functions:
        for bb in func.blocks:
            if not any(
                ins.sync_info is not None
                and ins.sync_info.on_wait is not None
                and len(ins.sync_info.on_wait) > 1
                for ins in bb.instructions
            ):
                continue
            new_insts = []
            for ins in bb.instructions:
                si = ins.sync_info
                if si is not None and si.on_wait is not None and len(si.on_wait) > 1:
                    waits = list(si.on_wait)
                    ups = list(si.on_update) if si.on_update is not None else []
                    for j, w in enumerate(waits[:-1]):
                        nop = mybir.InstNoOp(name=f"{ins.name}-wsp{j}")
                        nop.engine = ins.engine
                        nop.sync_info = bass_rust.SyncInfo(on_wait=[w], on_update=[])
                        new_insts.append(nop)
                        n += 1
                    ins.sync_info = bass_rust.SyncInfo(
                        on_wait=[waits[-1]], on_update=ups
                    )
                new_insts.append(ins)
            bb.instructions = new_insts
    return n


def _pack_k(w, nk, m):
    """(K, M) f32 -> (128, nk, M), row p of chunk k = w[k*128+p], zero pad."""
    k = w.shape[0]
    out = np.zeros((128, nk, m), np.float32)
    for c in range(nk):
        rows = min(128, k - c * 128)
        out[:rows, c, :] = w[c * 128 : c * 128 + rows, :]
    return out


def _pack_bias(b, nk):
    k = b.shape[0]
    out = np.zeros((128, nk), np.float32)
    for c in range(nk):
        rows = min(128, k - c * 128)
        out[:rows, c] = b[c * 128 : c * 128 + rows]
    return out


def _host_prep(inputs):
    f32 = np.float32
    x = np.asarray(inputs["x"], f32)
    ei = np.asarray(inputs["edge_index"]).astype(np.int64)
    batch = np.asarray(inputs["batch"]).astype(np.int64)
    sc_i = np.asarray(inputs["sc_i"]).astype(np.int64)
    sc_j = np.asarray(inputs["sc_j"]).astype(np.int64)
    sc_k = np.asarray(inputs["sc_k"]).astype(np.int64)
    sc_c = np.asarray(inputs["sc_c"], f32)
    kb_r = np.asarray(inputs["kb_r"]).astype(np.int64)
    kb_c = np.asarray(inputs["kb_c"]).astype(np.int64)
    kb_v = np.asarray(inputs["kb_v"], f32)

    src, dst = ei[0], ei[1]
    E = src.shape[0]
    owner = dst // S
    chunk = (dst % S) // CH

    # bucket edges by (core, chunk); uniform padded chunk capacity EC
    counts = np.zeros((NCORES, NCH), np.int64)
    np.add.at(counts, (owner, chunk), 1)
    EC = int(np.ceil(counts.max() / GG) * GG)
    EP = NCH * EC
    GC = EC // GG

    src_pad = np.zeros((NCORES, EP), np.int32)
    dst_loc = np.full((NCORES, EP), -1e9, f32)
    dst_pad = np.zeros((NCORES, EP), np.int32)
    order = np.lexsort((chunk, owner))
    so, sc_, ss, sd = owner[order], chunk[order], src[order], dst[order]
    pos = 0
    for c in range(NCORES):
        for ch in range(NCH):
            cnt = counts[c, ch]
            sl = slice(pos, pos + cnt)
            base = ch * EC
            src_pad[c, base : base + cnt] = ss[sl]
            dst_pad[c, base : base + cnt] = sd[sl]
            dst_loc[c, base : base + cnt] = (sd[sl] - (c * S + ch * CH)).astype(f32)
            pos += cnt

    # (128, EP/128) wrapped: column t holds edges t*128..t*128+127
    def wrap(a):
        return a.reshape(NCORES, EP // 128, 128).transpose(0, 2, 1).copy()

    src_w = wrap(src_pad)
    dst_w = wrap(dst_pad)
    dstloc_w = wrap(dst_loc)

    # per-core per-node-tile batch ids (pad -1)
    batchf = np.full((NCORES, 128, NT), -1.0, f32)
    for c in range(NCORES):
        for nt in range(NT):
            lo = nt * 128
            rows = min(128, S - lo)
            batchf[c, :rows, nt] = batch[c * S + lo : c * S + lo + rows].astype(f32)

    cnts = np.bincount(batch, minlength=NG).astype(f32)
    inv = 1.0 / np.maximum(cnts, 1.0)
    invrep = np.broadcast_to(inv[None, :], (128, NG)).copy()

    iota512 = np.broadcast_to(np.arange(512, dtype=f32)[None, :], (128, 512)).copy()
    iota128 = iota512[:, :128].copy()
    ident = np.eye(128, dtype=f32)

    # one-hot gather matrices and dense Killing matrix
    gi = np.zeros((D, NNZ), f32)
    gi[sc_i, np.arange(NNZ)] = 1.0
    gj = np.zeros((D, NNZ), f32)
    gj[sc_j, np.arange(NNZ)] = 1.0
    bmat = np.zeros((D, D), f32)
    np.add.at(bmat, (kb_r, kb_c), kb_v)

    w = {}
    w["wi1"] = _pack_k(np.asarray(inputs["Wi1"], f32), 1, HID).reshape(128, HID)
    w["bi1"] = _pack_bias(np.asarray(inputs["bi1"], f32), 2)
    w["wi2"] = _pack_k(np.asarray(inputs["Wi2"], f32), 2, D)
    w["bi2"] = _pack_bias(np.asarray(inputs["bi2"], f32), 2)
    msgW1 = np.asarray(inputs["msgW1"], f32)
    msgW2 = np.asarray(inputs["msgW2"], f32)
    updW1 = np.asarray(inputs["updW1"], f32)
    updW2 = np.asarray(inputs["updW2"], f32)
    w["msgW1a"] = np.stack([_pack_k(msgW1[l, :D], 2, HID) for l in range(NL)])
    w["wz"] = np.stack(
        [_pack_k(sc_c[:, None] * msgW1[l, D + sc_k], 4, HID) for l in range(NL)]
    )
    w["msgb1"] = np.stack([_pack_bias(np.asarray(inputs["msgb1"], f32)[l], 2) for l in range(NL)])
    w["msgW2"] = np.stack([_pack_k(msgW2[l], 2, D) for l in range(NL)])
    w["msgb2"] = np.stack([_pack_bias(np.asarray(inputs["msgb2"], f32)[l], 2) for l in range(NL)])
    w["updW1a"] = np.stack([_pack_k(updW1[l, :D], 2, HID) for l in range(NL)])
    w["updW1b"] = np.stack([_pack_k(updW1[l, D:], 2, HID) for l in range(NL)])
    w["updb1"] = np.stack([_pack_bias(np.asarray(inputs["updb1"], f32)[l], 2) for l in range(NL)])
    w["updW2"] = np.stack([_pack_k(updW2[l], 2, D) for l in range(NL)])
    w["updb2"] = np.stack([_pack_bias(np.asarray(inputs["updb2"], f32)[l], 2) for l in range(NL)])
    w["gi"] = _pack_k(gi, 2, NNZ)
    w["gj"] = _pack_k(gj, 2, NNZ)
    w["bmat"] = _pack_k(bmat, 2, D)
    w["wo1"] = _pack_k(np.asarray(inputs["Wo1"], f32), 2, HID)
    w["bo1"] = _pack_bias(np.asarray(inputs["bo1"], f32), 2)
    w["wo2"] = _pack_k(np.asarray(inputs["Wo2"], f32), 2, OUT)
    w["bo2"] = np.asarray(inputs["bo2"], f32).reshape(OUT, 1)

    shared = dict(w)
    shared["invrep"] = invrep
    shared["iota512"] = iota512
    shared["iota128"] = iota128
    shared["ident"] = ident

    per_core = []
    for c in range(NCORES):
        m = dict(shared)
        m["x"] = x[c * S : (c + 1) * S].copy()
        m["src32"] = src_w[c]
        m["dst32"] = dst_w[c]
        m["dstlocf"] = dstloc_w[c]
        m["batchf"] = batchf[c]
        per_core.append(m)
    return per_core, EC, GC


def _build_program(EC, GC):
    import concourse.bass as bass
    import concourse.tile as tile
    from concourse import mybir

    F32 = mybir.dt.float32
    CDT = mybir.dt.float32r
    I32 = mybir.dt.int32
    SILU = mybir.ActivationFunctionType.Silu
    ADD = mybir.AluOpType.add
    MULT = mybir.AluOpType.mult
    ISEQ = mybir.AluOpType.is_equal
    AX = mybir.AxisListType.X

    EP = NCH * EC
    EPC = EP // 128  # index columns

    nc = bass.Bass(num_devices=NCORES, target_bir_lowering=True)

    P = {}

    def param(name, shape, dt):
        P[name] = nc.declare_dram_parameter(name, list(shape), dt, isOutput=False)
        return P[name]

    param("x", (S, IN), CDT)
    param("src32", (128, EPC), I32)
    param("dst32", (128, EPC), I32)
    param("dstlocf", (128, EPC), F32)
    param("batchf", (128, NT), F32)
    param("invrep", (128, NG), F32)
    param("iota512", (128, 512), F32)
    param("iota128", (128, 128), F32)
    param("ident", (128, 128), CDT)
    param("wi1", (128, HID), CDT)
    param("bi1", (128, 2), F32)
    param("wi2", (128, 2, D), CDT)
    param("bi2", (128, 2), F32)
    param("msgW1a", (NL, 128, 2, HID), CDT)
    param("wz", (NL, 128, 4, HID), CDT)
    param("msgb1", (NL, 128, 2), F32)
    param("msgW2", (NL, 128, 2, D), CDT)
    param("msgb2", (NL, 128, 2), F32)
    param("updW1a", (NL, 128, 2, HID), CDT)
    param("updW1b", (NL, 128, 2, HID), CDT)
    param("updb1", (NL, 128, 2), F32)
    param("updW2", (NL, 128, 2, D), CDT)
    param("updb2", (NL, 128, 2), F32)
    param("gi", (128, 2, NNZ), CDT)
    param("gj", (128, 2, NNZ), CDT)
    param("bmat", (128, 2, D), CDT)
    param("wo1", (128, 2, HID), CDT)
    param("bo1", (128, 2), F32)
    param("wo2", (128, 2, OUT), CDT)
    param("bo2", (OUT, 1), F32)

    out_p = nc.declare_dram_parameter("out", [NG, OUT], F32, isOutput=True)

    h_shard = nc.dram_tensor("h_shard", [S, DP], CDT)
    h_full = nc.dram_tensor("h_full", [N, DP], CDT, addr_space="Shared")
    pooled_in = nc.dram_tensor("pooled_in", [256, NG], F32)
    pooled_out = nc.dram_tensor("pooled_out", [256, NG], F32, addr_space="Shared")

    RG = [list(range(NCORES))]

    with tile.TileContext(nc) as tc:
        with (
            tc.tile_pool(name="cpool", bufs=1) as cp,
            tc.tile_pool(name="state", bufs=1) as stp,
            tc.tile_pool(name="lw", bufs=2) as lwp,
        ):
            # ---- persistent constants ----
            c_t = {}
            for nm, shape, dt in [
                ("src32", (128, EPC), I32),
                ("dst32", (128, EPC), I32),
                ("dstlocf", (128, EPC), F32),
                ("batchf", (128, NT), F32),
                ("invrep", (128, NG), F32),
                ("iota512", (128, 512), F32),
                ("iota128", (128, 128), F32),
                ("ident", (128, 128), CDT),
                ("wi1", (128, HID), CDT),
                ("bi1", (128, 2), F32),
                ("wi2", (128, 2 * D), CDT),
                ("bi2", (128, 2), F32),
                ("gi", (128, 2 * NNZ), CDT),
                ("gj", (128, 2 * NNZ), CDT),
                ("bmat", (128, 2 * D), CDT),
                ("wo1", (128, 2 * HID), CDT),
                ("bo1", (128, 2), F32),
                ("wo2", (128, 2 * OUT), CDT),
                ("bo2", (OUT, 1), F32),
            ]:
                t = cp.tile(list(shape), dt, tag=nm)
                src_ap = P[nm][:]
                if len(P[nm].shape) == 3:
                    src_ap = src_ap.rearrange("p a b -> p (a b)")
                nc.sync.dma_start(out=t[:], in_=src_ap)
                c_t[nm] = t
            ident = c_t["ident"]

            # ---- state ----
            hT0 = stp.tile([128, S], F32, tag="hT0")
            hT1 = stp.tile([128, S], F32, tag="hT1")
            hTc0 = stp.tile([128, S], CDT, tag="hTc0")
            hTc1 = stp.tile([128, S], CDT, tag="hTc1")
            aggT0 = stp.tile([128, NCH * CH], CDT, tag="aggT0")
            aggT1 = stp.tile([128, NCH * CH], CDT, tag="aggT1")

            def load_layer_w(l):
                lw = {}
                for nm, cols in [
                    ("msgW1a", 2 * HID),
                    ("wz", 4 * HID),
                    ("msgW2", 2 * D),
                    ("updW1a", 2 * HID),
                    ("updW1b", 2 * HID),
                    ("updW2", 2 * D),
                ]:
                    t = lwp.tile([128, cols], CDT, tag=nm)
                    nc.sync.dma_start(
                        out=t[:], in_=P[nm][l].rearrange("p a b -> p (a b)")
                    )
                    lw[nm] = t
                for nm in ["msgb1", "msgb2", "updb1", "updb2"]:
                    t = lwp.tile([128, 2], F32, tag=nm)
                    nc.sync.dma_start(out=t[:], in_=P[nm][l])
                    lw[nm] = t
                return lw

            # ======== helper: update-style MLP over node chunks ========
            def node_chunk_cols(chi):
                lo = chi * CH
                return lo, min(CH, S - lo)

            def write_shard_tiles(src0, src1, psum_pool, work_pool):
                """transpose feature-major (CDT) h to node-major, DMA to h_shard"""
                for nt in range(NT):
                    lo = nt * 128
                    n_nodes = min(128, S - lo)
                    pt = psum_pool.tile([128, D], CDT, tag="nm_ps")
                    nc.tensor.matmul(
                        out=pt[:n_nodes, 0:128],
                        lhsT=src0[:, lo : lo + n_nodes],
                        rhs=ident[:, 0:128],
                        start=True, stop=True, is_transpose=True,
                    )
                    nc.tensor.matmul(
                        out=pt[:n_nodes, 128:D],
                        lhsT=src1[:120, lo : lo + n_nodes],
                        rhs=ident[:120, 0:120],
                        start=True, stop=True, is_transpose=True,
                    )
                    st = work_pool.tile([128, D], CDT, tag="nm_sb")
                    nc.vector.tensor_copy(out=st[:n_nodes], in_=pt[:n_nodes])
                    nc.sync.dma_start(
                        out=h_shard[lo : lo + n_nodes, 0:D], in_=st[:n_nodes]
                    )

            # ================= input MLP =================
            with (
                tc.tile_pool(name="in_sb", bufs=3) as isb,
                tc.tile_pool(name="in_ps", bufs=2, space="PSUM") as ips,
                tc.tile_pool(name="xT_pool", bufs=1) as xtp,
            ):
                xT = xtp.tile([128, S], CDT, tag="xT")
                for nt in range(NT):
                    lo = nt * 128
                    n_nodes = min(128, S - lo)
                    xt = isb.tile([128, IN], CDT, tag="x_t")
                    nc.sync.dma_start(out=xt[:n_nodes], in_=P["x"][lo : lo + n_nodes])
                    pt = ips.tile([128, 128], CDT, tag="xtp")
                    nc.tensor.matmul(
                        out=pt[:, :n_nodes],
                        lhsT=xt[:n_nodes],
                        rhs=ident[:n_nodes, :n_nodes],
                        start=True, stop=True, is_transpose=True,
                    )
                    nc.vector.tensor_copy(out=xT[:, lo : lo + n_nodes], in_=pt[:, :n_nodes])
                for chi in range(NCH):
                    lo, ncols = node_chunk_cols(chi)
                    us = []
                    for mc in range(2):
                        pu = ips.tile([128, CH], F32, tag="u_ps", space="PSUM")
                        nc.tensor.matmul(
                            out=pu[:, :ncols],
                            lhsT=c_t["wi1"][:, mc * 128 : (mc + 1) * 128],
                            rhs=xT[:, lo : lo + ncols],
                            start=True, stop=True,
                        )
                        u = isb.tile([128, CH], CDT, tag=f"u{mc}")
                        nc.scalar.activation(
                            out=u[:, :ncols], in_=pu[:, :ncols], func=SILU,
                            bias=c_t["bi1"][:, mc : mc + 1],
                        )
                        us.append(u)
                    for mc2 in range(2):
                        mw = 128 if mc2 == 0 else D - 128
                        ph = ips.tile([128, CH], F32, tag="h_ps", space="PSUM")
                        for kc in range(2):
                            nc.tensor.matmul(
                                out=ph[:mw, :ncols],
                                lhsT=c_t["wi2"][:, kc * D + mc2 * 128 : kc * D + mc2 * 128 + mw],
                                rhs=us[kc][:, :ncols],
                                start=(kc == 0), stop=(kc == 1),
                            )
                        hT = hT0 if mc2 == 0 else hT1
                        hTc = hTc0 if mc2 == 0 else hTc1
                        nc.vector.tensor_scalar_add(
                            out=hT[:mw, lo : lo + ncols], in0=ph[:mw, :ncols],
                            scalar1=c_t["bi2"][:mw, mc2 : mc2 + 1],
                        )
                        nc.vector.tensor_copy(
                            out=hTc[:mw, lo : lo + ncols], in_=hT[:mw, lo : lo + ncols]
                        )
                write_shard_tiles(hTc0, hTc1, ips, isb)
            nc.gpsimd.collective_compute(
                "AllGather", mybir.AluOpType.bypass, replica_groups=RG,
                ins=[h_shard[:].opt()], outs=[h_full[:].opt()],
            )

            # ================= message-passing layers =================
            for l in range(NL):
                lw = load_layer_w(l)
                with (
                    tc.tile_pool(name="e_sb", bufs=2) as esb,
                    tc.tile_pool(name="e_tp", bufs=2, space="PSUM") as ptp,
                    tc.tile_pool(name="e_g", bufs=1, space="PSUM") as pg,
                    tc.tile_pool(name="e_mh", bufs=1, space="PSUM") as pmh,
                    tc.tile_pool(name="e_mt", bufs=1, space="PSUM") as pmt,
                    tc.tile_pool(name="e_agg", bufs=1, space="PSUM") as pagg,
                ):
                    for chi in range(NCH):
                        aggP0 = pagg.tile([128, CH], F32, tag="aggP0", space="PSUM")
                        aggP1 = pagg.tile([128, CH], F32, tag="aggP1", space="PSUM")
                        for q in range(GC):
                            g = chi * GC + q
                            first = q == 0
                            last = q == GC - 1
                            # ---- gathers ----
                            hs_raw = esb.tile([128, 4, DP], CDT, tag="hs_raw")
                            hd_raw = esb.tile([128, 4, DP], CDT, tag="hd_raw")
                            for j in range(4):
                                col = g * 4 + j
                                nc.gpsimd.indirect_dma_start(
                                    out=hs_raw[:, j, :], out_offset=None,
                                    in_=h_full[:],
                                    in_offset=bass.IndirectOffsetOnAxis(
                                        ap=c_t["src32"][:, col : col + 1], axis=0),
                                )
                                nc.gpsimd.indirect_dma_start(
                                    out=hd_raw[:, j, :], out_offset=None,
                                    in_=h_full[:],
                                    in_offset=bass.IndirectOffsetOnAxis(
                                        ap=c_t["dst32"][:, col : col + 1], axis=0),
                                )
                            # ---- transposes to feature-major ----
                            hsT = []
                            for side, raw in (("s", hs_raw), ("d", hd_raw)):
                                t0 = esb.tile([128, GG], CDT, tag=f"h{side}T0")
                                t1 = esb.tile([128, GG], CDT, tag=f"h{side}T1")
                                for kc, tdst in ((0, t0), (1, t1)):
                                    kw = 128 if kc == 0 else D - 128
                                    pt = ptp.tile([128, GG], CDT, tag="tp", space="PSUM")
                                    for j in range(4):
                                        nc.tensor.matmul(
                                            out=pt[:kw, j * 128 : (j + 1) * 128],
                                            lhsT=hs_raw[:, j, kc * 128 : kc * 128 + kw] if side == "s" else hd_raw[:, j, kc * 128 : kc * 128 + kw],
                                            rhs=ident[:, 0:128],
                                            start=True, stop=True, is_transpose=True,
                                        )
                                    nc.vector.tensor_copy(out=tdst[:kw], in_=pt[:kw])
                                hsT.append((t0, t1))
                            (hsT0, hsT1), (hdT0, hdT1) = hsT
                            # ---- bracket T = (hs Gi) * (hd Gj), feature-major chunks ----
                            Ts = []
                            for zc in range(4):
                                pa = pg.tile([128, GG], F32, tag="hsG", space="PSUM")
                                pb = pg.tile([128, GG], F32, tag="hdG", space="PSUM")
                                for (gm, tt0, tt1, pp) in (
                                    ("gi", hsT0, hsT1, pa),
                                    ("gj", hdT0, hdT1, pb),
                                ):
                                    nc.tensor.matmul(
                                        out=pp[:],
                                        lhsT=c_t[gm][:, zc * 128 : (zc + 1) * 128],
                                        rhs=tt0[:],
                                        start=True, stop=False,
                                    )
                                    nc.tensor.matmul(
                                        out=pp[:],
                                        lhsT=c_t[gm][:120, NNZ + zc * 128 : NNZ + (zc + 1) * 128],
                                        rhs=tt1[:120],
                                        start=False, stop=True,
                                    )
                                sa = esb.tile([128, GG], F32, tag="hsG_sb")
                                nc.scalar.activation(
                                    out=sa[:], in_=pa[:],
                                    func=mybir.ActivationFunctionType.Copy,
                                )
                                tz = esb.tile([128, GG], CDT, tag=f"T{zc}")
                                nc.vector.tensor_tensor(out=tz[:], in0=sa[:], in1=pb[:], op=MULT)
                                Ts.append(tz)
                            # ---- message hidden + silu ----
                            us = []
                            for mc in range(2):
                                pu = pmh.tile([128, GG], F32, tag="mh", space="PSUM")
                                nc.tensor.matmul(
                                    out=pu[:], lhsT=lw["msgW1a"][:, 0 * HID + mc * 128 : 0 * HID + (mc + 1) * 128],
                                    rhs=hsT0[:], start=True, stop=False,
                                )
                                nc.tensor.matmul(
                                    out=pu[:], lhsT=lw["msgW1a"][:120, 1 * HID + mc * 128 : 1 * HID + (mc + 1) * 128],
                                    rhs=hsT1[:120], start=False, stop=False,
                                )
                                for zc in range(4):
                                    nc.tensor.matmul(
                                        out=pu[:],
                                        lhsT=lw["wz"][:, zc * HID + mc * 128 : zc * HID + (mc + 1) * 128],
                                        rhs=Ts[zc][:],
                                        start=False, stop=(zc == 3),
                                    )
                                u = esb.tile([128, GG], CDT, tag=f"mu{mc}")
                                nc.scalar.activation(
                                    out=u[:], in_=pu[:], func=SILU,
                                    bias=lw["msgb1"][:, mc : mc + 1],
                                )
                                us.append(u)
                            # ---- message out (feature-major) ----
                            mTs = []
                            for mc2 in range(2):
                                mw = 128 if mc2 == 0 else D - 128
                                pm = pmt.tile([128, GG], F32, tag="mt", space="PSUM")
                                for kc in range(2):
                                    nc.tensor.matmul(
                                        out=pm[:mw],
                                        lhsT=lw["msgW2"][:, kc * D + mc2 * 128 : kc * D + mc2 * 128 + mw],
                                        rhs=us[kc][:],
                                        start=(kc == 0), stop=(kc == 1),
                                    )
                                mt = esb.tile([128, GG], CDT, tag=f"mT{mc2}")
                                nc.vector.tensor_scalar_add(
                                    out=mt[:mw], in0=pm[:mw],
                                    scalar1=lw["msgb2"][:mw, mc2 : mc2 + 1],
                                )
                                mTs.append(mt)
                            # ---- back to edge-major + one-hot scatter ----
                            for j in range(4):
                                pe = ptp.tile([128, GG], CDT, tag="tp", space="PSUM")
                                nc.tensor.matmul(
                                    out=pe[:, 0:128],
                                    lhsT=mTs[0][:, j * 128 : (j + 1) * 128],
                                    rhs=ident[:, 0:128],
                                    start=True, stop=True, is_transpose=True,
                                )
                                nc.tensor.matmul(
                                    out=pe[:, 128:D],
                                    lhsT=mTs[1][:120, j * 128 : (j + 1) * 128],
                                    rhs=ident[:120, 0:120],
                                    start=True, stop=True, is_transpose=True,
                                )
                                me = esb.tile([128, D], CDT, tag="m_em")
                                nc.vector.tensor_copy(out=me[:], in_=pe[:, 0:D])
                                col = g * 4 + j
                                oh = esb.tile([128, 512], CDT, tag="oh")
                                nc.vector.tensor_tensor(
                                    out=oh[:],
                                    in0=c_t["dstlocf"][:, col : col + 1].to_broadcast([128, 512]),
                                    in1=c_t["iota512"][:],
                                    op=ISEQ,
                                )
                                nc.tensor.matmul(
                                    out=aggP0[:],
                                    lhsT=me[:, 0:128],
                                    rhs=oh[:],
                                    start=(first and j == 0), stop=(last and j == 3),
                                )
                                nc.tensor.matmul(
                                    out=aggP1[:120],
                                    lhsT=me[:, 128:D],
                                    rhs=oh[:],
                                    start=(first and j == 0), stop=(last and j == 3),
                                )
                        # evacuate chunk aggregation
                        lo = chi * CH
                        nc.vector.tensor_copy(out=aggT0[:, lo : lo + CH], in_=aggP0[:])
                        nc.vector.tensor_copy(out=aggT1[:120, lo : lo + CH], in_=aggP1[:120])

                # ---- update MLP ----
                with (
                    tc.tile_pool(name="u_sb", bufs=3) as usb,
                    tc.tile_pool(name="u_ps", bufs=2, space="PSUM") as ups,
                ):
                    for chi in range(NCH):
                        lo, ncols = node_chunk_cols(chi)
                        uus = []
                        for mc in range(2):
                            pu = ups.tile([128, CH], F32, tag="uh", space="PSUM")
                            nc.tensor.matmul(
                                out=pu[:, :ncols],
                                lhsT=lw["updW1a"][:, 0 * HID + mc * 128 : 0 * HID + (mc + 1) * 128],
                                rhs=hTc0[:, lo : lo + ncols], start=True, stop=False,
                            )
                            nc.tensor.matmul(
                                out=pu[:, :ncols],
                                lhsT=lw["updW1a"][:120, 1 * HID + mc * 128 : 1 * HID + (mc + 1) * 128],
                                rhs=hTc1[:120, lo : lo + ncols], start=False, stop=False,
                            )
                            nc.tensor.matmul(
                                out=pu[:, :ncols],
                                lhsT=lw["updW1b"][:, 0 * HID + mc * 128 : 0 * HID + (mc + 1) * 128],
                                rhs=aggT0[:, lo : lo + ncols], start=False, stop=False,
                            )
                            nc.tensor.matmul(
                                out=pu[:, :ncols],
                                lhsT=lw["updW1b"][:120, 1 * HID + mc * 128 : 1 * HID + (mc + 1) * 128],
                                rhs=aggT1[:120, lo : lo + ncols], start=False, stop=True,
                            )
                            uu = usb.tile([128, CH], CDT, tag=f"uu{mc}")
                            nc.scalar.activation(
                                out=uu[:, :ncols], in_=pu[:, :ncols], func=SILU,
                                bias=lw["updb1"][:, mc : mc + 1],
                            )
                            uus.append(uu)
                        for mc2 in range(2):
                            mw = 128 if mc2 == 0 else D - 128
                            pd = ups.tile([128, CH], F32, tag="d_ps", space="PSUM")
                            for kc in range(2):
                                nc.tensor.matmul(
                                    out=pd[:mw, :ncols],
                                    lhsT=lw["updW2"][:, kc * D + mc2 * 128 : kc * D + mc2 * 128 + mw],
                                    rhs=uus[kc][:, :ncols],
                                    start=(kc == 0), stop=(kc == 1),
                                )
                            hT = hT0 if mc2 == 0 else hT1
                            hTc = hTc0 if mc2 == 0 else hTc1
                            tmp = usb.tile([128, CH], F32, tag="d_tmp")
                            nc.vector.tensor_tensor(
                                out=tmp[:mw, :ncols], in0=pd[:mw, :ncols],
                                in1=hT[:mw, lo : lo + ncols], op=ADD,
                            )
                            nc.vector.tensor_scalar_add(
                                out=hT[:mw, lo : lo + ncols], in0=tmp[:mw, :ncols],
                                scalar1=lw["updb2"][:mw, mc2 : mc2 + 1],
                            )
                            nc.vector.tensor_copy(
                                out=hTc[:mw, lo : lo + ncols],
                                in_=hT[:mw, lo : lo + ncols],
                            )
                    write_shard_tiles(hTc0, hTc1, ups, usb)
                if l < NL - 1:
                    nc.gpsimd.collective_compute(
                        "AllGather", mybir.AluOpType.bypass, replica_groups=RG,
                        ins=[h_shard[:].opt()], outs=[h_full[:].opt()],
                    )

            # ================= readout =================
            with (
                tc.tile_pool(name="f_sb", bufs=3) as fsb,
                tc.tile_pool(name="f_ps", bufs=1, space="PSUM") as fps,
                tc.tile_pool(name="f_pool", bufs=1, space="PSUM") as fpl,
            ):
                pooled0 = fpl.tile([128, NG], F32, tag="pool0", space="PSUM")
                pooled1 = fpl.tile([128, NG], F32, tag="pool1", space="PSUM")
                for nt in range(NT):
                    lo = nt * 128
                    n_nodes = min(128, S - lo)
                    hn = fsb.tile([128, D], CDT, tag="h_nm")
                    nc.sync.dma_start(
                        out=hn[:n_nodes], in_=h_shard[lo : lo + n_nodes, 0:D]
                    )
                    # hB = h @ B (node-major rows)
                    pb = fps.tile([128, D], F32, tag="hB", space="PSUM")
                    nc.tensor.matmul(
                        out=pb[:n_nodes], lhsT=hTc0[:, lo : lo + n_nodes],
                        rhs=c_t["bmat"][:, 0:D], start=True, stop=False,
                    )
                    nc.tensor.matmul(
                        out=pb[:n_nodes], lhsT=hTc1[:120, lo : lo + n_nodes],
                        rhs=c_t["bmat"][:120, D : 2 * D], start=False, stop=True,
                    )
                    mu = fsb.tile([128, D], F32, tag="mu_f")
                    nc.vector.tensor_tensor(
                        out=mu[:n_nodes], in0=pb[:n_nodes],
                        in1=hn[:n_nodes].bitcast(F32), op=MULT,
                    )
                    ks = fsb.tile([128, 1], F32, tag="kscal")
                    nc.vector.reduce_sum(out=ks[:n_nodes], in_=mu[:n_nodes], axis=AX)
                    nf = fsb.tile([128, D + 1], CDT, tag="nf")
                    nc.vector.tensor_copy(out=nf[:n_nodes, 0:D], in_=hn[:n_nodes])
                    nc.vector.tensor_copy(out=nf[:n_nodes, D : D + 1], in_=ks[:n_nodes])
                    ohb = fsb.tile([128, NG], CDT, tag="ohb")
                    nc.vector.tensor_tensor(
                        out=ohb[:n_nodes],
                        in0=c_t["batchf"][:n_nodes, nt : nt + 1].to_broadcast([n_nodes, NG]),
                        in1=c_t["iota128"][:n_nodes],
                        op=ISEQ,
                    )
                    nc.tensor.matmul(
                        out=pooled0[:], lhsT=nf[:n_nodes, 0:128], rhs=ohb[:n_nodes],
                        start=(nt == 0), stop=(nt == NT - 1),
                    )
                    nc.tensor.matmul(
                        out=pooled1[: D + 1 - 128], lhsT=nf[:n_nodes, 128 : D + 1],
                        rhs=ohb[:n_nodes],
                        start=(nt == 0), stop=(nt == NT - 1),
                    )
                pl0 = fsb.tile([128, NG], F32, tag="pl0")
                pl1 = fsb.tile([128, NG], F32, tag="pl1")
                nc.vector.tensor_copy(out=pl0[:], in_=pooled0[:])
                nc.vector.tensor_copy(out=pl1[: D + 1 - 128], in_=pooled1[: D + 1 - 128])
                nc.sync.dma_start(out=pooled_in[0:128, :], in_=pl0[:])
                nc.sync.dma_start(out=pooled_in[128 : D + 1, :], in_=pl1[: D + 1 - 128])
                nc.gpsimd.collective_compute(
                    "AllReduce", ADD, replica_groups=RG,
                    ins=[pooled_in[:].opt()], outs=[pooled_out[:].opt()],
                )
                ps0 = fsb.tile([128, NG], CDT, tag="ps0")
                ps1 = fsb.tile([128, NG], CDT, tag="ps1")
                pr0 = fsb.tile([128, NG], F32, tag="pr0")
                pr1 = fsb.tile([128, NG], F32, tag="pr1")
                nc.sync.dma_start(out=pr0[:], in_=pooled_out[0:128, :])
                nc.sync.dma_start(out=pr1[: D + 1 - 128], in_=pooled_out[128 : D + 1, :])
                nc.vector.tensor_tensor(out=ps0[:], in0=pr0[:], in1=c_t["invrep"][:], op=MULT)
                nc.vector.tensor_tensor(
                    out=ps1[: D + 1 - 128], in0=pr1[: D + 1 - 128],
                    in1=c_t["invrep"][: D + 1 - 128], op=MULT,
                )
                s1s = []
                for mc in range(2):
                    po = fps.tile([128, NG], F32, tag="o1", space="PSUM")
                    nc.tensor.matmul(
                        out=po[:], lhsT=c_t["wo1"][:, 0 * HID + mc * 128 : 0 * HID + (mc + 1) * 128],
                        rhs=ps0[:], start=True, stop=False,
                    )
                    nc.tensor.matmul(
                        out=po[:], lhsT=c_t["wo1"][: D + 1 - 128, 1 * HID + mc * 128 : 1 * HID + (mc + 1) * 128],
                        rhs=ps1[: D + 1 - 128], start=False, stop=True,
                    )
                    s1 = fsb.tile([128, NG], CDT, tag=f"s1{mc}")
                    nc.scalar.activation(
                        out=s1[:], in_=po[:], func=SILU, bias=c_t["bo1"][:, mc : mc + 1]
                    )
                    s1s.append(s1)
                po2 = fps.tile([OUT, NG], F32, tag="o2", space="PSUM")
                for kc in range(2):
                    nc.tensor.matmul(
                        out=po2[:], lhsT=c_t["wo2"][:, kc * OUT : (kc + 1) * OUT],
                        rhs=s1s[kc][:], start=(kc == 0), stop=(kc == 1),
                    )
                o2 = fsb.tile([OUT, NG], CDT, tag="o2sb")
                nc.vector.tensor_scalar_add(out=o2[:], in0=po2[:], scalar1=c_t["bo2"][:, 0:1])
                pot = fps.tile([128, OUT], CDT, tag="ot", space="PSUM")
                nc.tensor.matmul(
                    out=pot[:], lhsT=o2[:], rhs=ident[:OUT, :OUT],
                    start=True, stop=True, is_transpose=True,
                )
                of = fsb.tile([128, OUT], F32, tag="of")
                nc.vector.tensor_copy(out=of[:], in_=pot[:])
                nc.sync.dma_start(out=out_p[:], in_=of[:])

    return nc


def _run(inputs, trace=False, tmpdir=None):
    import sys
    if "/opt/trn_rl_repo" not in sys.path:
        sys.path.insert(0, "/opt/trn_rl_repo")
    import bass_rust
    from concourse import mybir
    from concourse.bass_utils import run_bass_kernel_spmd

    per_core, EC, GC = _host_prep(inputs)
    nc = _build_program(EC, GC)
    _split_multi_waits(nc, mybir, bass_rust)
    res = run_bass_kernel_spmd(
        nc, per_core, list(range(NCORES)), trace=trace, tmpdir=tmpdir
    )
    return np.asarray(res.results[0]["out"], np.float32), res


def kernel(**inputs) -> np.ndarray:
    out, _ = _run(inputs)
    return out
